# revision 10
# baseline (speedup 1.0000x reference)
"""Trainium2 Bass kernel for nn_Block_40742059770386 (dense_cnn).

Per-sample adaptively-mixed, style-modulated, demodulated 3x3 conv
(StyleGAN2-style) + channel RMS norm + SiLU.

Sharding: data-parallel over batch. B=16 samples -> 8 cores x 2 samples.
The small kernel bank (2 x 256 x 256 x 3 x 3) and gamma are replicated.

Key reformulation (avoids materializing per-sample demodulated weights):
    w       = (a0*W0 + a1*W1) * (mod+1)[i] * d[o]
    y_conv  = conv(x_mod, a0*W0 + a1*W1),  x_mod = x * (mod+1)[i]
    d[o]    = rsqrt(max(sum_i m2[i] * sum_kk wmix[o,i,kk]^2, eps)),
              m2[i] = (mod[i]+1)^2     (tiny fp32 matvec on TensorE)
    nsum[p] = sum_o (d[o]*y_conv[o,p])^2 = sum_o d[o]^2 * y_conv[o,p]^2
              (matmul with lhsT = d^2 column; row per pixel-tile in PSUM)
    out     = silu(y_conv * inv_norm[p] * (d[o]*gamma[o]*sqrt(256)))

Engine notes:
  - conv = implicit GEMM over zero-padded x [128, 66, 66] with shifted APs;
    fp32r matmuls (full PE rate at N=512); every fp32r operand is produced
    by a compute op with float32r output dtype (walrus requires rounding).
  - ACT functions limited to {Square, Sqrt, Sigmoid} with Sqrt batched
    once per sample to minimize the 1283ns activation-table reloads.
  - demod rsqrt done DVE-only via bit-trick seed + 2 Newton steps.
  - y_conv staged via DRAM (PSUM->DRAM->SBUF) so PSUM banks free per
    pixel-tile while the channel-norm is batched per sample.
"""

import os
import numpy as np

import concourse.bass as bass
import concourse.bacc as bacc
import concourse.mybir as mybir
import concourse.tile as tile
from contextlib import ExitStack
from concourse.bass_utils import run_bass_kernel_spmd

# ---- problem constants (hardcoded; kernel.py must be self-contained) ----
B, C_IN, C_OUT, H, W, K, NK = 16, 256, 256, 64, 64, 3, 2
N_CORES = 8
S = B // N_CORES            # samples per core
PB = 128                    # partitions per block
IB = C_IN // PB             # input channel blocks
OB = C_OUT // PB            # output channel blocks
HW = H * W                  # 4096
PADH, PADW = H + 2, W + 2   # 66, 66
PT = 512                    # pixels per tile (one PSUM bank of fp32)
ROWS_PT = PT // W           # 8 rows per pixel tile
NPT = HW // PT              # 8 pixel tiles
KK = K * K                  # 9
OCH = 64                    # o-chunk for weight demod stats
EPS = 1e-8

F32 = mybir.dt.float32
F32R = mybir.dt.float32r
BF16 = mybir.dt.bfloat16
I32 = mybir.dt.int32

# "f32r": full-rate near-fp32 matmuls; "bf16": bf16 matmuls
MM_MODE = os.environ.get("KERNEL_MM_MODE", "f32r")

AF = mybir.ActivationFunctionType
ALU = mybir.AluOpType
MAGIC = 0x5F3759DF


def _newton_rsqrt_steps(nc, pool, r, x, shape, tag, iters):
    """Refine r ~ rsqrt(x): r' = r * (1.5 - 0.5 * x * r^2). Returns tile."""
    xh = pool.tile(shape, F32, tag=f"{tag}_xh", name=f"{tag}_xh")
    nc.vector.tensor_scalar_mul(out=xh, in0=x, scalar1=0.5)
    for it in range(iters):
        t = pool.tile(shape, F32, tag=f"{tag}_t", name=f"{tag}_t")
        nc.vector.tensor_mul(out=t, in0=r, in1=r)
        nc.vector.tensor_mul(out=t, in0=t, in1=xh)
        nc.vector.tensor_scalar(
            out=t, in0=t, scalar1=-1.0, scalar2=1.5, op0=ALU.mult, op1=ALU.add
        )
        r2 = pool.tile(shape, F32, tag=f"{tag}_r", name=f"{tag}_r")
        nc.vector.tensor_mul(out=r2, in0=r, in1=t)
        r = r2
    return r


def _rsqrt_dve(nc, pool, src_ap, clamp, shape, tag):
    """rsqrt(max(src, clamp)) entirely on DVE: bit-trick seed + 2 Newton."""
    x = pool.tile(shape, F32, tag=f"{tag}_x", name=f"{tag}_x")
    nc.vector.tensor_scalar_max(out=x, in0=src_ap, scalar1=float(clamp))
    seed = pool.tile(shape, I32, tag=f"{tag}_s", name=f"{tag}_s")
    nc.vector.tensor_scalar(
        out=seed, in0=x.bitcast(I32), scalar1=1, scalar2=None,
        op0=ALU.logical_shift_right,
    )                                   # bits >> 1
    nc.vector.tensor_scalar(
        out=seed, in0=seed, scalar1=-1, scalar2=MAGIC,
        op0=ALU.mult, op1=ALU.add,
    )                                   # MAGIC - (bits >> 1)
    return _newton_rsqrt_steps(nc, pool, seed.bitcast(F32), x, shape, tag, iters=2)


def _rsqrt_act(nc, pool, src_ap, clamp, shape, tag):
    """rsqrt(max(src, clamp)) via ACT Sqrt + DVE recip + 1 Newton step."""
    x = pool.tile(shape, F32, tag=f"{tag}_x", name=f"{tag}_x")
    nc.vector.tensor_scalar_max(out=x, in0=src_ap, scalar1=float(clamp))
    r = pool.tile(shape, F32, tag=f"{tag}_r0", name=f"{tag}_r0")
    nc.scalar.activation(out=r, in_=x, func=AF.Sqrt)
    nc.vector.reciprocal(out=r, in_=r)
    return _newton_rsqrt_steps(nc, pool, r, x, shape, tag, iters=1)


def build_program(mm_mode=MM_MODE):
    nc = bacc.Bacc(trn_type="TRN2", debug=False)

    x_d = nc.declare_dram_parameter("x", [S, IB, PB, HW], F32, isOutput=False)
    wt_d = nc.declare_dram_parameter("wT", [NK, IB, PB, C_OUT, KK], F32, isOutput=False)
    attn_d = nc.declare_dram_parameter("attn", [S, NK, PB, 1], F32, isOutput=False)
    modp1_d = nc.declare_dram_parameter("modp1", [S, IB, PB, 1], F32, isOutput=False)
    m2_d = nc.declare_dram_parameter("m2", [S, IB, PB, 1], F32, isOutput=False)
    g16_d = nc.declare_dram_parameter("g16", [OB, PB, 1], F32, isOutput=False)
    y_d = nc.declare_dram_parameter("y", [S, OB, PB, HW], F32, isOutput=True)

    mm_dt = {"f32r": F32R, "bf16": BF16}[mm_mode]

    with ExitStack() as ctx:
        tc = ctx.enter_context(tile.TileContext(nc))
        const = ctx.enter_context(tc.tile_pool(name="const", bufs=1))
        wpool = ctx.enter_context(tc.tile_pool(name="wmix", bufs=3))
        wsqp = ctx.enter_context(tc.tile_pool(name="wsq", bufs=2))
        xfp = ctx.enter_context(tc.tile_pool(name="xf32", bufs=2))
        xrp = ctx.enter_context(tc.tile_pool(name="xpad", bufs=3))
        small = ctx.enter_context(tc.tile_pool(name="small", bufs=4))
        rows = ctx.enter_context(tc.tile_pool(name="rows", bufs=1))
        sq_p = ctx.enter_context(tc.tile_pool(name="ycsq", bufs=3))
        outp = ctx.enter_context(tc.tile_pool(name="outs", bufs=2))
        bcastp = ctx.enter_context(tc.tile_pool(name="bcast", bufs=2))
        dramp = ctx.enter_context(tc.tile_pool(name="dram", bufs=2, space="DRAM"))
        pconv = ctx.enter_context(tc.tile_pool(name="pconv", bufs=5, space="PSUM"))
        pnorm = ctx.enter_context(tc.tile_pool(name="pnorm", bufs=2, space="PSUM"))
        pdsq = ctx.enter_context(tc.tile_pool(name="pdsq", bufs=1, space="PSUM"))

        # ---- resident constants ----
        wbank = [
            [const.tile([PB, C_OUT, KK], F32, tag=f"wb{n}{ib}", name=f"wb{n}{ib}")
             for ib in range(IB)]
            for n in range(NK)
        ]
        for n in range(NK):
            for ib in range(IB):
                nc.sync.dma_start(out=wbank[n][ib], in_=wt_d[n, ib])
        g16sb = [const.tile([PB, 1], F32, tag=f"g16_{ob}", name=f"g16_{ob}")
                 for ob in range(OB)]
        for ob in range(OB):
            nc.sync.dma_start(out=g16sb[ob], in_=g16_d[ob])

        for s in range(S):
            # ---- tiny per-sample vectors ----
            acol = [small.tile([PB, 1], F32, tag=f"a{n}", name=f"a{n}")
                    for n in range(NK)]
            for n in range(NK):
                nc.sync.dma_start(out=acol[n], in_=attn_d[s, n])
            mpc = [small.tile([PB, 1], F32, tag=f"mp{ib}", name=f"mp{ib}")
                   for ib in range(IB)]
            m2c = [small.tile([PB, 1], F32, tag=f"m2{ib}", name=f"m2{ib}")
                   for ib in range(IB)]
            for ib in range(IB):
                nc.sync.dma_start(out=mpc[ib], in_=modp1_d[s, ib])
                nc.sync.dma_start(out=m2c[ib], in_=m2_d[s, ib])

            # ---- mix kernel bank (rounded to matmul dtype) + demod stats ----
            dsq_ps = [pdsq.tile([PB, 1], F32, tag="dsq", name="dsq")
                      for _ in range(OB)]
            wmix = []
            for ib in range(IB):
                wm = wpool.tile([PB, C_OUT, KK], mm_dt, tag="wmix", name="wmix")
                nc.vector.tensor_scalar_mul(out=wm, in0=wbank[0][ib], scalar1=acol[0])
                nc.vector.scalar_tensor_tensor(
                    out=wm, in0=wbank[1][ib], scalar=acol[1], in1=wm,
                    op0=ALU.mult, op1=ALU.add,
                )
                wmix.append(wm)
                # per-(i,o) sum over kk of wmix^2, chunked over o to save SBUF
                wsr = wsqp.tile([PB, C_OUT, 1], F32, tag="wsr", name="wsr")
                for c in range(C_OUT // OCH):
                    wsq = wsqp.tile([PB, OCH, KK], F32, tag="wsq", name="wsq")
                    nc.scalar.activation(
                        out=wsq, in_=wm[:, c * OCH:(c + 1) * OCH, :], func=AF.Square
                    )
                    nc.vector.tensor_reduce(
                        out=wsr[:, c * OCH:(c + 1) * OCH, :], in_=wsq,
                        axis=mybir.AxisListType.X, op=ALU.add,
                    )
                # dsq[o] += wsr[:, o_blk].T @ m2  (exact fp32 matvec, tiny)
                for ob in range(OB):
                    nc.tensor.matmul(
                        dsq_ps[ob],
                        lhsT=wsr[:, ob * PB:(ob + 1) * PB, 0],
                        rhs=m2c[ib],
                        start=(ib == 0),
                        stop=(ib == IB - 1),
                    )

            # d = rsqrt(max(dsq, EPS)) on DVE; d2 (matmul dtype); gd = d*g16
            d2col, gdcol = [], []
            for ob in range(OB):
                d = _rsqrt_dve(nc, small, dsq_ps[ob], EPS, [PB, 1], f"d{ob}")
                d2 = small.tile([PB, 1], mm_dt, tag=f"d2_{ob}", name=f"d2_{ob}")
                nc.vector.tensor_mul(out=d2, in0=d, in1=d)
                d2col.append(d2)
                gd = small.tile([PB, 1], F32, tag=f"gd{ob}", name=f"gd{ob}")
                nc.vector.tensor_mul(out=gd, in0=d, in1=g16sb[ob])
                gdcol.append(gd)

            # ---- x: load, modulate by (mod+1), zero-padded + rounded ----
            xp = []
            for ib in range(IB):
                xf = xfp.tile([PB, PADH, PADW], F32, tag="xf32", name="xf32")
                nc.gpsimd.memset(xf[:, 0:1, :], 0.0)
                nc.gpsimd.memset(xf[:, PADH - 1:PADH, :], 0.0)
                nc.gpsimd.memset(xf[:, 1:H + 1, 0:1], 0.0)
                nc.gpsimd.memset(xf[:, 1:H + 1, PADW - 1:PADW], 0.0)
                nc.sync.dma_start(
                    out=xf[:, 1:H + 1, 1:W + 1],
                    in_=x_d[s, ib].rearrange("p (h w) -> p h w", w=W),
                )
                xr = xrp.tile([PB, PADH, PADW], mm_dt, tag="xpad", name="xpad")
                nc.vector.tensor_scalar_mul(out=xr, in0=xf, scalar1=mpc[ib])
                xp.append(xr)

            # ---- conv (implicit GEMM); square + stage y_conv to DRAM ----
            ycd = dramp.tile([OB, NPT, PB, PT], F32, tag="ycd", name="ycd")
            dgath = dramp.tile([NPT, PT], F32, tag="dgath", name="dgath")
            for pt in range(NPT):
                nsum_ps = pnorm.tile([1, PT], F32, tag="nsum", name="nsum")
                for ob in range(OB):
                    ps = pconv.tile([PB, PT], F32, tag="conv", name="conv")
                    n_mm = IB * KK
                    i_mm = 0
                    for ib in range(IB):
                        for ki in range(K):
                            for kj in range(K):
                                lhsT = wmix[ib][:, ob * PB:(ob + 1) * PB, ki * K + kj]
                                rhs = xp[ib][
                                    :,
                                    pt * ROWS_PT + ki: pt * ROWS_PT + ki + ROWS_PT,
                                    kj: kj + W,
                                ]
                                nc.tensor.matmul(
                                    ps, lhsT=lhsT, rhs=rhs,
                                    start=(i_mm == 0), stop=(i_mm == n_mm - 1),
                                )
                                i_mm += 1
                    sq = sq_p.tile([PB, PT], mm_dt, tag="ycsq", name="ycsq")
                    nc.scalar.activation(out=sq, in_=ps, func=AF.Square)
                    ycs = sq_p.tile([PB, PT], F32, tag="ycs", name="ycs")
                    nc.scalar.activation(out=ycs, in_=ps, func=AF.Copy)
                    nc.sync.dma_start(out=ycd[ob, pt], in_=ycs)
                    # nsum row for this pixel tile
                    nc.tensor.matmul(
                        nsum_ps,
                        lhsT=d2col[ob][:], rhs=sq[:],
                        start=(ob == 0), stop=(ob == OB - 1),
                    )
                nrow = rows.tile([1, PT], F32, tag="nrow", name="nrow")
                nc.scalar.activation(out=nrow, in_=nsum_ps, func=AF.Copy)
                nc.sync.dma_start(out=dgath[pt], in_=nrow)

            # ---- batched channel-norm + SiLU epilogue ----
            gath = rows.tile([NPT, PT], F32, tag="gath", name="gath")
            nc.sync.dma_start(out=gath, in_=dgath)
            inv8 = _rsqrt_act(nc, rows, gath, 1e-24, [NPT, PT], "inv")
            dinv = dramp.tile([NPT, PT], F32, tag="dinv", name="dinv")
            nc.sync.dma_start(out=dinv, in_=inv8)
            for pt in range(NPT):
                invb = bcastp.tile([PB, PT], F32, tag="invb", name="invb")
                nc.sync.dma_start(out=invb, in_=dinv[pt:pt + 1, :].to_broadcast((PB, PT)))
                for ob in range(OB):
                    yd = outp.tile([PB, PT], F32, tag="yd", name="yd")
                    nc.sync.dma_start(out=yd, in_=ycd[ob, pt])
                    z = outp.tile([PB, PT], F32, tag="z", name="z")
                    nc.vector.scalar_tensor_tensor(
                        out=z, in0=yd, scalar=gdcol[ob], in1=invb,
                        op0=ALU.mult, op1=ALU.mult,
                    )
                    sg = outp.tile([PB, PT], F32, tag="sg", name="sg")
                    nc.scalar.activation(out=sg, in_=z, func=AF.Sigmoid)
                    yo = outp.tile([PB, PT], F32, tag="yo", name="yo")
                    nc.vector.tensor_mul(out=yo, in0=z, in1=sg)
                    nc.sync.dma_start(out=y_d[s, ob, :, pt * PT:(pt + 1) * PT], in_=yo)
    nc.finalize()
    return nc


_NC_CACHE = {}


def _get_program(mm_mode=MM_MODE):
    if mm_mode not in _NC_CACHE:
        _NC_CACHE[mm_mode] = build_program(mm_mode)
    return _NC_CACHE[mm_mode]


def _host_prep(x, mod, kernel_mod, weights, gamma):
    x = np.asarray(x, dtype=np.float32)
    mod = np.asarray(mod, dtype=np.float32)
    kernel_mod = np.asarray(kernel_mod, dtype=np.float32)
    weights = np.asarray(weights, dtype=np.float32)
    gamma = np.asarray(gamma, dtype=np.float32)

    # softmax over the (tiny) kernel bank dim
    e = np.exp(kernel_mod - kernel_mod.max(axis=-1, keepdims=True))
    attn = (e / e.sum(axis=-1, keepdims=True)).astype(np.float32)     # [B, NK]

    modp1 = mod + 1.0                                                 # [B, C_IN]
    m2 = modp1 * modp1

    # [NK, O, I, K, K] -> [NK, I, O, K*K] -> [NK, IB, PB, C_OUT, KK]
    wT = np.ascontiguousarray(
        weights.transpose(0, 2, 1, 3, 4).reshape(NK, IB, PB, C_OUT, KK)
    )
    g16 = np.ascontiguousarray(
        (gamma * np.sqrt(C_OUT)).astype(np.float32).reshape(OB, PB, 1)
    )

    in_maps = []
    for c in range(N_CORES):
        sl = slice(c * S, (c + 1) * S)
        in_maps.append({
            "x": np.ascontiguousarray(x[sl].reshape(S, IB, PB, HW)),
            "wT": wT,
            "attn": np.ascontiguousarray(
                np.broadcast_to(attn[sl][:, :, None, None], (S, NK, PB, 1))
            ),
            "modp1": np.ascontiguousarray(modp1[sl].reshape(S, IB, PB, 1)),
            "m2": np.ascontiguousarray(m2[sl].reshape(S, IB, PB, 1)),
            "g16": g16,
        })
    return in_maps


def kernel(x, mod, kernel_mod, weights, gamma, _trace=False, _trace_kwargs=None):
    nc = _get_program()
    in_maps = _host_prep(x, mod, kernel_mod, weights, gamma)
    res = run_bass_kernel_spmd(
        nc, in_maps, list(range(N_CORES)),
        trace=_trace, **(_trace_kwargs or {}),
    )
    y = np.concatenate(
        [res.results[c]["y"].reshape(S, C_OUT, H, W) for c in range(N_CORES)],
        axis=0,
    ).astype(np.float32)
    if _trace:
        kernel.last_results = res
    return y


kernel.last_results = None
